# revision 18
# baseline (speedup 1.0000x reference)
"""ConvCaps (routing-by-agreement) Trainium2 kernel, v1.

Problem: pose (4, 512, 32, 32) f32, W (288, 512, 16) f32 ->
         out (4, 512, 15, 15) f32.

Per (b,l) position (900 total, padded to 1024 = 8 cores x 128):
  p[l]   : (288, 16) unfolded poses       (k = kk*32 + a)
  votes  : V[k, bd] = sum_c W[k, bd, c] * p[l, k, c]     (288, 512)
  3 routing iters (softmax over B=32 caps, D=16), output v_3 (l, 512).

v1 strategy vs v0:
  - All matmuls in bf16 (FWL weight loads, 1 col/cycle streaming).
  - W and p SBUF-resident in ONE fused tensor, loaded once:
    PW2[p=(ksub8,c16), ch36, 640] where [:512]=W cols (d,B)-major,
    [512:]=pT positions. Serves both the dense pass-1 (contraction
    (k,c)=128/chunk) and per-k vote matmuls (16-deep slices).
  - Pass 1 (uniform c): s1 = (1/32)*sum_kc p*W as 36 dense matmuls.
  - Votes emitted (d,B)-major so every big DVE op keeps an innermost
    step-1 bf16 axis -> 2x_1p perf mode; reductions done as pairwise
    trees (2x) instead of tensor_reduce (1x).
  - Engine balance: ACT does PSUM->SBUF casts + exp, gpsimd takes
    u-tree L1 + c-normalize + s-accumulate, DVE the rest.
"""

import numpy as np
import ml_dtypes

import concourse.bass as bass
import concourse.tile as tile
from concourse import mybir
from concourse.bass_utils import run_bass_kernel_spmd
from concourse.vector_clock import ScopedClock

# ---- problem constants ----
A, B, K, P, STRIDE, ITERS = 32, 32, 3, 4, 2, 3
C = P * P            # 16
D = P * P            # 16
KK = K * K           # 9
KKA = KK * A         # 288
BD = B * D           # 512
EPS = 1e-8
H = W_IN = 32
OH = (H - K) // STRIDE + 1   # 15
OW = OH                      # 15
L = OH * OW                  # 225
NB = 4                       # batch
NPOS = NB * L                # 900
NCORES = 8
LP = 128                     # positions per core (padded)
NPOS_PAD = NCORES * LP       # 1024

G = 4                        # k-group size (PSUM banks per votes tile)
NG = KKA // G                # 72 groups
NCH = KKA // 8               # 36 chunks of 8 k's (128 = 8k x 16c rows)

F32 = mybir.dt.float32
F32R = mybir.dt.float32r
BF16 = mybir.dt.bfloat16

AX = mybir.AxisListType
OP = mybir.AluOpType
ACT = mybir.ActivationFunctionType


class _ChunkedDrainTileContext(tile.TileContext):
    """Work around a walrus limit of 2 sem-waits per CTRL instruction:
    split the kernel-tail drain's waits across per-processor drains."""

    def _drain_and_barrier(self, tick_clock, wait_clock):
        vclock = tick_clock.global_clock
        observed = ScopedClock()
        for i in range(len(vclock)):
            if vclock[i] > 0:
                partial = ScopedClock()
                partial.require_at_least(None, i, vclock[i])
                d = self.nc.sync.drain()
                wait_clock.add_sem_waits(d.ins, partial, observed)
                observed.update_past(partial)
        drain_inst = self.nc.sync.drain()
        wait_clock.add_sem_waits(
            drain_inst.ins, ScopedClock({None: tick_clock.global_clock}), observed
        )
        self.nc.all_engine_barrier()
        assert self.sems is not None
        popped = self.nc._tile_sem_poison_stack.pop()
        assert popped is self._sem_poison
        self.nc.clear_and_free_semaphores(list(self.sems.allocated().values()))
        self.nc.all_engine_barrier()


def _squash(nc, pool, ss, eps_t, tag):
    """ss: [128, 512] f32 SBUF, (d,B)-major. returns v [128, 512] f32."""
    sq = pool.tile([LP, BD], F32, tag=f"sq{tag}")
    nc.scalar.square(out=sq, in_=ss)
    n2 = pool.tile([LP, B], F32, tag=f"n2{tag}")
    # reduce over d (stride 32): view [p, b, d] then reduce innermost
    nc.vector.tensor_reduce(
        out=n2, in_=sq.rearrange("p (d b) -> p b d", d=D), axis=AX.X, op=OP.add
    )
    # (n2+eps)^-1/2 = exp(-0.5*ln(n2+eps))
    lg = pool.tile([LP, B], F32, tag=f"lg{tag}")
    nc.scalar.activation(out=lg, in_=n2, func=ACT.Ln, bias=eps_t, scale=1.0)
    rs = pool.tile([LP, B], F32, tag=f"rs{tag}")
    nc.scalar.activation(out=rs, in_=lg, func=ACT.Exp, bias=0.0, scale=-0.5)
    np1 = pool.tile([LP, B], F32, tag=f"np1{tag}")
    nc.vector.tensor_scalar_add(out=np1, in0=n2, scalar1=1.0)
    rnp1 = pool.tile([LP, B], F32, tag=f"rnp1{tag}")
    nc.vector.reciprocal(out=rnp1, in_=np1)
    f1 = pool.tile([LP, B], F32, tag=f"f1{tag}")
    nc.vector.tensor_mul(out=f1, in0=n2, in1=rs)
    fac = pool.tile([LP, B], F32, tag=f"fac{tag}")
    nc.vector.tensor_mul(out=fac, in0=f1, in1=rnp1)
    v = pool.tile([LP, BD], F32, tag=f"v{tag}")
    nc.vector.tensor_mul(
        out=v.rearrange("p (d b) -> p d b", d=D),
        in0=ss.rearrange("p (d b) -> p d b", d=D),
        in1=fac.unsqueeze(1).to_broadcast([LP, D, B]),
    )
    return v


def _build_nc():
    nc = bass.Bass("TRN2", target_bir_lowering=False, debug=False)
    PW_d = nc.dram_tensor("PW", [C, KKA, BD + LP], BF16, kind="ExternalInput")
    out_d = nc.dram_tensor("vout", [LP, BD], F32, kind="ExternalOutput")
    PW = PW_d.ap()
    vout = out_d.ap()

    with _ChunkedDrainTileContext(nc) as tc:
        import contextlib

        with contextlib.ExitStack() as ctx:
            keep = ctx.enter_context(tc.tile_pool(name="keep", bufs=1))
            wring = ctx.enter_context(tc.tile_pool(name="wring", bufs=6))
            big = ctx.enter_context(tc.tile_pool(name="big", bufs=6))
            small = ctx.enter_context(tc.tile_pool(name="small", bufs=6))
            sq_pool = ctx.enter_context(tc.tile_pool(name="sqp", bufs=2))

            eps_t = keep.tile([LP, 1], F32, tag="eps")
            nc.vector.memset(eps_t, EPS)

            def load_group(g):
                """stream [16, G, 640] W||p tile for G k's (base partition 0)."""
                t = wring.tile([C, G, BD + LP], BF16, tag="wt")
                nc.sync.dma_start(out=t, in_=PW[:, g * G:(g + 1) * G, :])
                return t

            # ---------- pass 1: s1 = (1/32) sum_k V (streamed, PSUM-accum) ----------
            with tc.tile_pool(name="s1", bufs=1, space="PSUM") as s1_pool:
                psum_s = s1_pool.tile([LP, BD], F32)
                for g in range(NG):
                    wt = load_group(g)
                    for j in range(G):
                        k = g * G + j
                        nc.tensor.matmul(
                            psum_s,
                            lhsT=wt[:, j, BD:BD + LP],
                            rhs=wt[:, j, 0:BD],
                            start=(k == 0),
                            stop=(k == KKA - 1),
                        )
                ss1 = keep.tile([LP, BD], F32, tag="ss1")
                nc.scalar.mul(out=ss1, in_=psum_s, mul=1.0 / B)
            w = _squash(nc, small, ss1, eps_t, "i0")  # v1; w for iter 2

            vp_pool = ctx.enter_context(
                tc.tile_pool(name="vp", bufs=2, space="PSUM")
            )

            # ---------- iters 2..3 ----------
            for it in range(1, ITERS):
                wB = keep.tile([LP, BD], BF16, tag=f"wB{it}")
                nc.scalar.copy(out=wB, in_=w)
                wBb = wB.unsqueeze(1).to_broadcast([LP, G, BD])
                s_acc = keep.tile([LP, BD], F32, tag=f"sacc{it}")
                nc.vector.memset(s_acc, 0.0)

                def front(g):
                    """PE votes -> ACT cast -> DVE um -> gp t1. Returns (vt, t1)."""
                    wt = load_group(g)
                    vp = vp_pool.tile([LP, G, BD], F32, tag="vp")
                    for j in range(G):
                        nc.tensor.matmul(
                            vp[:, j, :],
                            lhsT=wt[:, j, BD:BD + LP],
                            rhs=wt[:, j, 0:BD],
                            start=True,
                            stop=True,
                        )
                    vt = big.tile([LP, G, BD], BF16, tag="vt")
                    nc.scalar.copy(out=vt, in_=vp)
                    um = big.tile([LP, G, BD], BF16, tag="um")
                    for j in range(G):
                        nc.vector.tensor_mul(
                            out=um[:, j], in0=vt[:, j], in1=wB
                        )
                    um4 = um.rearrange("p g (d b) -> p g d b", d=D)
                    t1 = big.tile([LP, G, 8, B], BF16, tag="t1")
                    nc.gpsimd.tensor_add(
                        out=t1, in0=um4[:, :, 0:8, :], in1=um4[:, :, 8:16, :]
                    )
                    return vt, t1

                def tail(vt, t1):
                    """finish softmax + weighted sum for a deferred group."""
                    t2 = small.tile([LP, G, 4, B], BF16, tag="t2")
                    nc.gpsimd.tensor_add(
                        out=t2, in0=t1[:, :, 0:4, :], in1=t1[:, :, 4:8, :]
                    )
                    t3 = small.tile([LP, G, 2, B], BF16, tag="t3")
                    nc.vector.tensor_add(
                        out=t3, in0=t2[:, :, 0:2, :], in1=t2[:, :, 2:4, :]
                    )
                    u = small.tile([LP, G, B], BF16, tag="u")
                    nc.vector.tensor_add(
                        out=u, in0=t3[:, :, 0, :], in1=t3[:, :, 1, :]
                    )
                    e = small.tile([LP, G, B], F32, tag="e")
                    nc.scalar.activation(out=e, in_=u, func=ACT.Exp)
                    Z = small.tile([LP, G], F32, tag="Z")
                    nc.vector.tensor_reduce(out=Z, in_=e, axis=AX.X, op=OP.add)
                    rZ = small.tile([LP, G], F32, tag="rZ")
                    nc.vector.reciprocal(out=rZ, in_=Z)
                    c = small.tile([LP, G, B], BF16, tag="c")
                    nc.gpsimd.tensor_mul(
                        out=c, in0=e, in1=rZ.unsqueeze(2).to_broadcast([LP, G, B])
                    )
                    cv = big.tile([LP, G, D, B], BF16, tag="cv")
                    vt4 = vt.rearrange("p g (d b) -> p g d b", d=D)
                    for j in range(G):
                        nc.vector.tensor_mul(
                            out=cv[:, j],
                            in0=vt4[:, j],
                            in1=c[:, j].unsqueeze(1).to_broadcast([LP, D, B]),
                        )
                    cvf = cv.rearrange("p g d b -> p (g d b)").rearrange(
                        "p (h x) -> p h x", h=2
                    )
                    st = small.tile([LP, 2 * BD], BF16, tag="st")
                    nc.vector.tensor_add(out=st, in0=cvf[:, 0, :], in1=cvf[:, 1, :])
                    sp = small.tile([LP, BD], BF16, tag="sp")
                    nc.gpsimd.tensor_add(
                        out=sp, in0=st[:, 0:BD], in1=st[:, BD:2 * BD]
                    )
                    nc.gpsimd.tensor_add(out=s_acc, in0=s_acc, in1=sp)

                DEFER = 2
                pend = []
                for g in range(NG):
                    pend.append(front(g))
                    if len(pend) > DEFER:
                        tail(*pend.pop(0))
                for pb in pend:
                    tail(*pb)

                v_it = _squash(nc, sq_pool, s_acc, eps_t, f"i{it}")
                if it < ITERS - 1:
                    w_new = keep.tile([LP, BD], F32, tag=f"w{it}")
                    nc.vector.tensor_add(out=w_new, in0=w, in1=v_it)
                    w = w_new
                else:
                    nc.sync.dma_start(out=vout[:, :], in_=v_it)
    _split_excess_waits(nc)
    return nc


def _host_prep(pose, W):
    """unfold + reorder + shard. returns in_maps."""
    pose = np.asarray(pose, dtype=np.float32)
    W = np.asarray(W, dtype=np.float32)
    b = pose.shape[0]
    cols = np.empty((b, A * C, KK, OH, OW), dtype=np.float32)
    for ki in range(K):
        for kj in range(K):
            cols[:, :, ki * K + kj] = pose[
                :, :, ki:ki + STRIDE * (OH - 1) + 1:STRIDE,
                kj:kj + STRIDE * (OW - 1) + 1:STRIDE,
            ]
    # (b, A, C, KK, l) -> (b, l, KK, A, C) -> (npos, KKA, C)
    p = cols.reshape(b, A, C, KK, L).transpose(0, 4, 3, 1, 2).reshape(
        NPOS, KKA, C
    )
    p_pad = np.zeros((NPOS_PAD, KKA, C), dtype=np.float32)
    p_pad[:NPOS] = p
    # W cols (d,B)-major: Wd[k, c, d*32+B] = W[k, B*16+d, c]
    Wd = W.reshape(KKA, B, D, C).transpose(0, 3, 2, 1).reshape(KKA, C, D * B)
    # streaming layout [16c, 288k, 512]
    Wt = Wd.transpose(1, 0, 2).astype(ml_dtypes.bfloat16)
    in_maps = []
    for i in range(NCORES):
        pc = p_pad[i * LP:(i + 1) * LP]                    # [LP, 288, 16]
        PWs = np.empty((C, KKA, BD + LP), dtype=ml_dtypes.bfloat16)
        PWs[:, :, :BD] = Wt
        PWs[:, :, BD:] = pc.transpose(2, 1, 0).astype(ml_dtypes.bfloat16)
        in_maps.append({"PW": PWs})
    return in_maps


def _gather(results):
    v = np.concatenate([r["vout"] for r in results], axis=0)  # [1024, 512] (d,B)
    v = v[:NPOS].reshape(NB, L, D, B).transpose(0, 3, 2, 1)   # -> (NB, B, D, L)
    return np.ascontiguousarray(
        v.reshape(NB, BD, OH, OW), dtype=np.float32
    )


def _split_excess_waits(nc, max_waits=1):
    """walrus (CoreV2/V3) accepts at most 2 sync-wait commands per
    compute instruction and 1 per DMA; hoist excess waits onto NOPs
    just before, same engine."""
    n_split = 0
    for f in nc.m.functions:
        for bb in f.blocks:
            il = bb.instructions
            out = []
            changed = False
            for inst in il:
                lim = max_waits
                si = inst.sync_info
                if si is not None and si.on_wait and len(si.on_wait) > lim:
                    waits = list(si.on_wait)
                    excess, kept = waits[:-lim], waits[-lim:]
                    for i in range(0, len(excess), max_waits):
                        nop = mybir.InstNoOp(
                            name=f"{inst.name}-w{i}",
                            sync_info=mybir.SyncInfo(
                                on_wait=excess[i:i + max_waits], on_update=[]
                            ),
                            bass_nofuse=True,
                            engine=inst.engine,
                        )
                        out.append(nop)
                        n_split += 1
                    inst.sync_info = mybir.SyncInfo(
                        on_wait=kept, on_update=list(si.on_update or [])
                    )
                    changed = True
                out.append(inst)
            if changed:
                bb.instructions = out
    return n_split


_NC_CACHE = {}


def _get_nc(mm_dtype=None):
    key = "v1"
    if key not in _NC_CACHE:
        _NC_CACHE[key] = _build_nc()
    return _NC_CACHE[key]


def _run(pose, W, trace=False, mm_dtype=None):
    nc = _get_nc(mm_dtype)
    in_maps = _host_prep(pose, W)
    res = run_bass_kernel_spmd(
        nc, in_maps, core_ids=list(range(NCORES)), trace=trace
    )
    return _gather(res.results), res


def kernel(pose, W):
    out, _ = _run(pose, W)
    return out


# revision 19
# speedup vs baseline: 1.1422x; 1.1422x over previous
"""ConvCaps (routing-by-agreement) Trainium2 kernel, v1.

Problem: pose (4, 512, 32, 32) f32, W (288, 512, 16) f32 ->
         out (4, 512, 15, 15) f32.

Per (b,l) position (900 total, padded to 1024 = 8 cores x 128):
  p[l]   : (288, 16) unfolded poses       (k = kk*32 + a)
  votes  : V[k, bd] = sum_c W[k, bd, c] * p[l, k, c]     (288, 512)
  3 routing iters (softmax over B=32 caps, D=16), output v_3 (l, 512).

v1 strategy vs v0:
  - All matmuls in bf16 (FWL weight loads, 1 col/cycle streaming).
  - W and p SBUF-resident in ONE fused tensor, loaded once:
    PW2[p=(ksub8,c16), ch36, 640] where [:512]=W cols (d,B)-major,
    [512:]=pT positions. Serves both the dense pass-1 (contraction
    (k,c)=128/chunk) and per-k vote matmuls (16-deep slices).
  - Pass 1 (uniform c): s1 = (1/32)*sum_kc p*W as 36 dense matmuls.
  - Votes emitted (d,B)-major so every big DVE op keeps an innermost
    step-1 bf16 axis -> 2x_1p perf mode; reductions done as pairwise
    trees (2x) instead of tensor_reduce (1x).
  - Engine balance: ACT does PSUM->SBUF casts + exp, gpsimd takes
    u-tree L1 + c-normalize + s-accumulate, DVE the rest.
"""

import numpy as np
import ml_dtypes

import concourse.bass as bass
import concourse.tile as tile
from concourse import mybir
from concourse.bass_utils import run_bass_kernel_spmd
from concourse.vector_clock import ScopedClock

# ---- problem constants ----
A, B, K, P, STRIDE, ITERS = 32, 32, 3, 4, 2, 3
C = P * P            # 16
D = P * P            # 16
KK = K * K           # 9
KKA = KK * A         # 288
BD = B * D           # 512
EPS = 1e-8
H = W_IN = 32
OH = (H - K) // STRIDE + 1   # 15
OW = OH                      # 15
L = OH * OW                  # 225
NB = 4                       # batch
NPOS = NB * L                # 900
NCORES = 8
LP = 128                     # positions per core (padded)
NPOS_PAD = NCORES * LP       # 1024

G = 4                        # k-group size (PSUM banks per votes tile)
NG = KKA // G                # 72 groups
NCH = KKA // 8               # 36 chunks of 8 k's (128 = 8k x 16c rows)

F32 = mybir.dt.float32
F32R = mybir.dt.float32r
BF16 = mybir.dt.bfloat16

AX = mybir.AxisListType
OP = mybir.AluOpType
ACT = mybir.ActivationFunctionType


class _ChunkedDrainTileContext(tile.TileContext):
    """Work around a walrus limit of 2 sem-waits per CTRL instruction:
    split the kernel-tail drain's waits across per-processor drains."""

    def _drain_and_barrier(self, tick_clock, wait_clock):
        vclock = tick_clock.global_clock
        observed = ScopedClock()
        for i in range(len(vclock)):
            if vclock[i] > 0:
                partial = ScopedClock()
                partial.require_at_least(None, i, vclock[i])
                d = self.nc.sync.drain()
                wait_clock.add_sem_waits(d.ins, partial, observed)
                observed.update_past(partial)
        drain_inst = self.nc.sync.drain()
        wait_clock.add_sem_waits(
            drain_inst.ins, ScopedClock({None: tick_clock.global_clock}), observed
        )
        self.nc.all_engine_barrier()
        assert self.sems is not None
        popped = self.nc._tile_sem_poison_stack.pop()
        assert popped is self._sem_poison
        self.nc.clear_and_free_semaphores(list(self.sems.allocated().values()))
        self.nc.all_engine_barrier()


def _squash(nc, pool, ss, eps_t, tag):
    """ss: [128, 512] f32 SBUF, (d,B)-major. returns v [128, 512] f32."""
    sq = pool.tile([LP, BD], F32, tag=f"sq{tag}")
    nc.scalar.square(out=sq, in_=ss)
    n2 = pool.tile([LP, B], F32, tag=f"n2{tag}")
    # reduce over d (stride 32): view [p, b, d] then reduce innermost
    nc.vector.tensor_reduce(
        out=n2, in_=sq.rearrange("p (d b) -> p b d", d=D), axis=AX.X, op=OP.add
    )
    # (n2+eps)^-1/2 = exp(-0.5*ln(n2+eps))
    lg = pool.tile([LP, B], F32, tag=f"lg{tag}")
    nc.scalar.activation(out=lg, in_=n2, func=ACT.Ln, bias=eps_t, scale=1.0)
    rs = pool.tile([LP, B], F32, tag=f"rs{tag}")
    nc.scalar.activation(out=rs, in_=lg, func=ACT.Exp, bias=0.0, scale=-0.5)
    np1 = pool.tile([LP, B], F32, tag=f"np1{tag}")
    nc.vector.tensor_scalar_add(out=np1, in0=n2, scalar1=1.0)
    rnp1 = pool.tile([LP, B], F32, tag=f"rnp1{tag}")
    nc.vector.reciprocal(out=rnp1, in_=np1)
    f1 = pool.tile([LP, B], F32, tag=f"f1{tag}")
    nc.vector.tensor_mul(out=f1, in0=n2, in1=rs)
    fac = pool.tile([LP, B], F32, tag=f"fac{tag}")
    nc.vector.tensor_mul(out=fac, in0=f1, in1=rnp1)
    v = pool.tile([LP, BD], F32, tag=f"v{tag}")
    nc.vector.tensor_mul(
        out=v.rearrange("p (d b) -> p d b", d=D),
        in0=ss.rearrange("p (d b) -> p d b", d=D),
        in1=fac.unsqueeze(1).to_broadcast([LP, D, B]),
    )
    return v


def _build_nc():
    nc = bass.Bass("TRN2", target_bir_lowering=False, debug=False)
    PW_d = nc.dram_tensor("PW", [C, KKA, BD + LP], BF16, kind="ExternalInput")
    out_d = nc.dram_tensor("vout", [LP, BD], F32, kind="ExternalOutput")
    PW = PW_d.ap()
    vout = out_d.ap()

    with _ChunkedDrainTileContext(nc) as tc:
        import contextlib

        with contextlib.ExitStack() as ctx:
            keep = ctx.enter_context(tc.tile_pool(name="keep", bufs=1))
            wring = ctx.enter_context(tc.tile_pool(name="wring", bufs=6))
            big = ctx.enter_context(tc.tile_pool(name="big", bufs=6))
            small = ctx.enter_context(tc.tile_pool(name="small", bufs=6))
            sq_pool = ctx.enter_context(tc.tile_pool(name="sqp", bufs=2))

            eps_t = keep.tile([LP, 1], F32, tag="eps")
            nc.vector.memset(eps_t, EPS)

            def load_group(g):
                """stream [16, G, 640] W||p tile for G k's (base partition 0)."""
                t = wring.tile([C, G, BD + LP], BF16, tag="wt")
                nc.sync.dma_start(out=t, in_=PW[:, g * G:(g + 1) * G, :])
                return t

            # ---------- pass 1: s1 = (1/32) sum_k V (streamed, PSUM-accum) ----------
            with tc.tile_pool(name="s1", bufs=1, space="PSUM") as s1_pool:
                psum_s = s1_pool.tile([LP, BD], F32)
                for g in range(NG):
                    wt = load_group(g)
                    for j in range(G):
                        k = g * G + j
                        nc.tensor.matmul(
                            psum_s,
                            lhsT=wt[:, j, BD:BD + LP],
                            rhs=wt[:, j, 0:BD],
                            start=(k == 0),
                            stop=(k == KKA - 1),
                        )
                ss1 = keep.tile([LP, BD], F32, tag="ss1")
                nc.scalar.mul(out=ss1, in_=psum_s, mul=1.0 / B)
            w = _squash(nc, small, ss1, eps_t, "i0")  # v1; w for iter 2

            vp_pool = ctx.enter_context(
                tc.tile_pool(name="vp", bufs=2, space="PSUM")
            )

            # ---------- iters 2..3 ----------
            for it in range(1, ITERS):
                wB = keep.tile([LP, BD], BF16, tag=f"wB{it}")
                nc.scalar.copy(out=wB, in_=w)
                wBb = wB.unsqueeze(1).to_broadcast([LP, G, BD])
                s_acc = keep.tile([LP, BD], F32, tag=f"sacc{it}")
                nc.vector.memset(s_acc, 0.0)

                def front(g):
                    """PE votes -> ACT cast -> DVE um -> gp t1. Returns (vt, t1)."""
                    wt = load_group(g)
                    vp = vp_pool.tile([LP, G, BD], F32, tag="vp")
                    for j in range(G):
                        nc.tensor.matmul(
                            vp[:, j, :],
                            lhsT=wt[:, j, BD:BD + LP],
                            rhs=wt[:, j, 0:BD],
                            start=True,
                            stop=True,
                        )
                    vt = big.tile([LP, G, BD], BF16, tag="vt")
                    nc.scalar.copy(out=vt, in_=vp)
                    um = big.tile([LP, G, BD], BF16, tag="um")
                    for j in range(G):
                        nc.vector.tensor_mul(
                            out=um[:, j], in0=vt[:, j], in1=wB
                        )
                    um4 = um.rearrange("p g (d b) -> p g d b", d=D)
                    t1 = big.tile([LP, G, 8, B], BF16, tag="t1")
                    nc.gpsimd.tensor_add(
                        out=t1, in0=um4[:, :, 0:8, :], in1=um4[:, :, 8:16, :]
                    )
                    return vt, t1

                def tail(vt, t1):
                    """finish softmax + weighted sum for a deferred group."""
                    t2 = small.tile([LP, G, 4, B], BF16, tag="t2")
                    nc.vector.tensor_add(
                        out=t2, in0=t1[:, :, 0:4, :], in1=t1[:, :, 4:8, :]
                    )
                    t3 = small.tile([LP, G, 2, B], BF16, tag="t3")
                    nc.vector.tensor_add(
                        out=t3, in0=t2[:, :, 0:2, :], in1=t2[:, :, 2:4, :]
                    )
                    u = small.tile([LP, G, B], BF16, tag="u")
                    nc.vector.tensor_add(
                        out=u, in0=t3[:, :, 0, :], in1=t3[:, :, 1, :]
                    )
                    e = small.tile([LP, G, B], F32, tag="e")
                    nc.scalar.activation(out=e, in_=u, func=ACT.Exp)
                    Z = small.tile([LP, G], F32, tag="Z")
                    nc.vector.tensor_reduce(out=Z, in_=e, axis=AX.X, op=OP.add)
                    rZ = small.tile([LP, G], F32, tag="rZ")
                    nc.vector.reciprocal(out=rZ, in_=Z)
                    c = small.tile([LP, G, B], BF16, tag="c")
                    nc.gpsimd.tensor_mul(
                        out=c, in0=e, in1=rZ.unsqueeze(2).to_broadcast([LP, G, B])
                    )
                    cv = big.tile([LP, G, D, B], BF16, tag="cv")
                    vt4 = vt.rearrange("p g (d b) -> p g d b", d=D)
                    for j in range(G):
                        nc.vector.tensor_mul(
                            out=cv[:, j],
                            in0=vt4[:, j],
                            in1=c[:, j].unsqueeze(1).to_broadcast([LP, D, B]),
                        )
                    cvf = cv.rearrange("p g d b -> p (g d b)").rearrange(
                        "p (h x) -> p h x", h=2
                    )
                    st = small.tile([LP, 2 * BD], BF16, tag="st")
                    nc.vector.tensor_add(out=st, in0=cvf[:, 0, :], in1=cvf[:, 1, :])
                    sp = small.tile([LP, BD], BF16, tag="sp")
                    nc.vector.tensor_add(
                        out=sp, in0=st[:, 0:BD], in1=st[:, BD:2 * BD]
                    )
                    nc.gpsimd.tensor_add(out=s_acc, in0=s_acc, in1=sp)

                DEFER = 2
                pend = []
                for g in range(NG):
                    pend.append(front(g))
                    if len(pend) > DEFER:
                        tail(*pend.pop(0))
                for pb in pend:
                    tail(*pb)

                v_it = _squash(nc, sq_pool, s_acc, eps_t, f"i{it}")
                if it < ITERS - 1:
                    w_new = keep.tile([LP, BD], F32, tag=f"w{it}")
                    nc.vector.tensor_add(out=w_new, in0=w, in1=v_it)
                    w = w_new
                else:
                    nc.sync.dma_start(out=vout[:, :], in_=v_it)
    _split_excess_waits(nc)
    return nc


def _host_prep(pose, W):
    """unfold + reorder + shard. returns in_maps."""
    pose = np.asarray(pose, dtype=np.float32)
    W = np.asarray(W, dtype=np.float32)
    b = pose.shape[0]
    cols = np.empty((b, A * C, KK, OH, OW), dtype=np.float32)
    for ki in range(K):
        for kj in range(K):
            cols[:, :, ki * K + kj] = pose[
                :, :, ki:ki + STRIDE * (OH - 1) + 1:STRIDE,
                kj:kj + STRIDE * (OW - 1) + 1:STRIDE,
            ]
    # (b, A, C, KK, l) -> (b, l, KK, A, C) -> (npos, KKA, C)
    p = cols.reshape(b, A, C, KK, L).transpose(0, 4, 3, 1, 2).reshape(
        NPOS, KKA, C
    )
    p_pad = np.zeros((NPOS_PAD, KKA, C), dtype=np.float32)
    p_pad[:NPOS] = p
    # W cols (d,B)-major: Wd[k, c, d*32+B] = W[k, B*16+d, c]
    Wd = W.reshape(KKA, B, D, C).transpose(0, 3, 2, 1).reshape(KKA, C, D * B)
    # streaming layout [16c, 288k, 512]
    Wt = Wd.transpose(1, 0, 2).astype(ml_dtypes.bfloat16)
    in_maps = []
    for i in range(NCORES):
        pc = p_pad[i * LP:(i + 1) * LP]                    # [LP, 288, 16]
        PWs = np.empty((C, KKA, BD + LP), dtype=ml_dtypes.bfloat16)
        PWs[:, :, :BD] = Wt
        PWs[:, :, BD:] = pc.transpose(2, 1, 0).astype(ml_dtypes.bfloat16)
        in_maps.append({"PW": PWs})
    return in_maps


def _gather(results):
    v = np.concatenate([r["vout"] for r in results], axis=0)  # [1024, 512] (d,B)
    v = v[:NPOS].reshape(NB, L, D, B).transpose(0, 3, 2, 1)   # -> (NB, B, D, L)
    return np.ascontiguousarray(
        v.reshape(NB, BD, OH, OW), dtype=np.float32
    )


def _split_excess_waits(nc, max_waits=1):
    """walrus (CoreV2/V3) accepts at most 2 sync-wait commands per
    compute instruction and 1 per DMA; hoist excess waits onto NOPs
    just before, same engine."""
    n_split = 0
    for f in nc.m.functions:
        for bb in f.blocks:
            il = bb.instructions
            out = []
            changed = False
            for inst in il:
                lim = max_waits
                si = inst.sync_info
                if si is not None and si.on_wait and len(si.on_wait) > lim:
                    waits = list(si.on_wait)
                    excess, kept = waits[:-lim], waits[-lim:]
                    for i in range(0, len(excess), max_waits):
                        nop = mybir.InstNoOp(
                            name=f"{inst.name}-w{i}",
                            sync_info=mybir.SyncInfo(
                                on_wait=excess[i:i + max_waits], on_update=[]
                            ),
                            bass_nofuse=True,
                            engine=inst.engine,
                        )
                        out.append(nop)
                        n_split += 1
                    inst.sync_info = mybir.SyncInfo(
                        on_wait=kept, on_update=list(si.on_update or [])
                    )
                    changed = True
                out.append(inst)
            if changed:
                bb.instructions = out
    return n_split


_NC_CACHE = {}


def _get_nc(mm_dtype=None):
    key = "v1"
    if key not in _NC_CACHE:
        _NC_CACHE[key] = _build_nc()
    return _NC_CACHE[key]


def _run(pose, W, trace=False, mm_dtype=None):
    nc = _get_nc(mm_dtype)
    in_maps = _host_prep(pose, W)
    res = run_bass_kernel_spmd(
        nc, in_maps, core_ids=list(range(NCORES)), trace=trace
    )
    return _gather(res.results), res


def kernel(pose, W):
    out, _ = _run(pose, W)
    return out


# revision 20
# speedup vs baseline: 1.3198x; 1.1555x over previous
"""ConvCaps (routing-by-agreement) Trainium2 kernel, v1.

Problem: pose (4, 512, 32, 32) f32, W (288, 512, 16) f32 ->
         out (4, 512, 15, 15) f32.

Per (b,l) position (900 total, padded to 1024 = 8 cores x 128):
  p[l]   : (288, 16) unfolded poses       (k = kk*32 + a)
  votes  : V[k, bd] = sum_c W[k, bd, c] * p[l, k, c]     (288, 512)
  3 routing iters (softmax over B=32 caps, D=16), output v_3 (l, 512).

v1 strategy vs v0:
  - All matmuls in bf16 (FWL weight loads, 1 col/cycle streaming).
  - W and p SBUF-resident in ONE fused tensor, loaded once:
    PW2[p=(ksub8,c16), ch36, 640] where [:512]=W cols (d,B)-major,
    [512:]=pT positions. Serves both the dense pass-1 (contraction
    (k,c)=128/chunk) and per-k vote matmuls (16-deep slices).
  - Pass 1 (uniform c): s1 = (1/32)*sum_kc p*W as 36 dense matmuls.
  - Votes emitted (d,B)-major so every big DVE op keeps an innermost
    step-1 bf16 axis -> 2x_1p perf mode; reductions done as pairwise
    trees (2x) instead of tensor_reduce (1x).
  - Engine balance: ACT does PSUM->SBUF casts + exp, gpsimd takes
    u-tree L1 + c-normalize + s-accumulate, DVE the rest.
"""

import numpy as np
import ml_dtypes

import concourse.bass as bass
import concourse.tile as tile
from concourse import mybir
from concourse.bass_utils import run_bass_kernel_spmd
from concourse.vector_clock import ScopedClock

# ---- problem constants ----
A, B, K, P, STRIDE, ITERS = 32, 32, 3, 4, 2, 3
C = P * P            # 16
D = P * P            # 16
KK = K * K           # 9
KKA = KK * A         # 288
BD = B * D           # 512
EPS = 1e-8
H = W_IN = 32
OH = (H - K) // STRIDE + 1   # 15
OW = OH                      # 15
L = OH * OW                  # 225
NB = 4                       # batch
NPOS = NB * L                # 900
NCORES = 8
LP = 128                     # positions per core (padded)
NPOS_PAD = NCORES * LP       # 1024

G = 4                        # k-group size (PSUM banks per votes tile)
NG = KKA // G                # 72 groups
NCH = KKA // 8               # 36 chunks of 8 k's (128 = 8k x 16c rows)

F32 = mybir.dt.float32
F32R = mybir.dt.float32r
BF16 = mybir.dt.bfloat16

AX = mybir.AxisListType
OP = mybir.AluOpType
ACT = mybir.ActivationFunctionType


class _ChunkedDrainTileContext(tile.TileContext):
    """Work around a walrus limit of 2 sem-waits per CTRL instruction:
    split the kernel-tail drain's waits across per-processor drains."""

    def _drain_and_barrier(self, tick_clock, wait_clock):
        vclock = tick_clock.global_clock
        observed = ScopedClock()
        for i in range(len(vclock)):
            if vclock[i] > 0:
                partial = ScopedClock()
                partial.require_at_least(None, i, vclock[i])
                d = self.nc.sync.drain()
                wait_clock.add_sem_waits(d.ins, partial, observed)
                observed.update_past(partial)
        drain_inst = self.nc.sync.drain()
        wait_clock.add_sem_waits(
            drain_inst.ins, ScopedClock({None: tick_clock.global_clock}), observed
        )
        self.nc.all_engine_barrier()
        assert self.sems is not None
        popped = self.nc._tile_sem_poison_stack.pop()
        assert popped is self._sem_poison
        self.nc.clear_and_free_semaphores(list(self.sems.allocated().values()))
        self.nc.all_engine_barrier()


def _squash(nc, pool, ss, eps_t, tag):
    """ss: [128, 512] f32 SBUF, (d,B)-major. returns v [128, 512] f32."""
    sq = pool.tile([LP, BD], F32, tag=f"sq{tag}")
    nc.scalar.square(out=sq, in_=ss)
    n2 = pool.tile([LP, B], F32, tag=f"n2{tag}")
    # reduce over d (stride 32): view [p, b, d] then reduce innermost
    nc.vector.tensor_reduce(
        out=n2, in_=sq.rearrange("p (d b) -> p b d", d=D), axis=AX.X, op=OP.add
    )
    # (n2+eps)^-1/2 = exp(-0.5*ln(n2+eps))
    lg = pool.tile([LP, B], F32, tag=f"lg{tag}")
    nc.scalar.activation(out=lg, in_=n2, func=ACT.Ln, bias=eps_t, scale=1.0)
    rs = pool.tile([LP, B], F32, tag=f"rs{tag}")
    nc.scalar.activation(out=rs, in_=lg, func=ACT.Exp, bias=0.0, scale=-0.5)
    np1 = pool.tile([LP, B], F32, tag=f"np1{tag}")
    nc.vector.tensor_scalar_add(out=np1, in0=n2, scalar1=1.0)
    rnp1 = pool.tile([LP, B], F32, tag=f"rnp1{tag}")
    nc.vector.reciprocal(out=rnp1, in_=np1)
    f1 = pool.tile([LP, B], F32, tag=f"f1{tag}")
    nc.vector.tensor_mul(out=f1, in0=n2, in1=rs)
    fac = pool.tile([LP, B], F32, tag=f"fac{tag}")
    nc.vector.tensor_mul(out=fac, in0=f1, in1=rnp1)
    v = pool.tile([LP, BD], F32, tag=f"v{tag}")
    nc.vector.tensor_mul(
        out=v.rearrange("p (d b) -> p d b", d=D),
        in0=ss.rearrange("p (d b) -> p d b", d=D),
        in1=fac.unsqueeze(1).to_broadcast([LP, D, B]),
    )
    return v


def _build_nc():
    nc = bass.Bass("TRN2", target_bir_lowering=False, debug=False)
    PW_d = nc.dram_tensor("PW", [C, KKA, BD + LP], BF16, kind="ExternalInput")
    PW2_d = nc.dram_tensor("PW2", [LP, NCH, BD + LP], BF16, kind="ExternalInput")
    out_d = nc.dram_tensor("vout", [LP, BD], F32, kind="ExternalOutput")
    PW = PW_d.ap()
    PW2 = PW2_d.ap()
    vout = out_d.ap()

    with _ChunkedDrainTileContext(nc) as tc:
        import contextlib

        with contextlib.ExitStack() as ctx:
            keep = ctx.enter_context(tc.tile_pool(name="keep", bufs=1))
            wring = ctx.enter_context(tc.tile_pool(name="wring", bufs=4))
            big = ctx.enter_context(tc.tile_pool(name="big", bufs=3))
            small = ctx.enter_context(tc.tile_pool(name="small", bufs=4))
            sq_pool = ctx.enter_context(tc.tile_pool(name="sqp", bufs=2))

            eps_t = keep.tile([LP, 1], F32, tag="eps")
            nc.vector.memset(eps_t, EPS)

            def load_group(g):
                """stream [16, G, 640] W||p tile for G k's (base partition 0)."""
                t = wring.tile([C, G, BD + LP], BF16, tag="wt")
                nc.sync.dma_start(out=t, in_=PW[:, g * G:(g + 1) * G, :])
                return t

            # ---------- pass 1: s1 = (1/32) sum_k V  (dense (k,c)) ----------
            ws = keep.tile([LP, NCH, BD + LP], BF16, tag="ws")
            nc.sync.dma_start(out=ws, in_=PW2[:, :, :])
            with tc.tile_pool(name="s1", bufs=1, space="PSUM") as s1_pool:
                psum_s = s1_pool.tile([LP, BD], F32)
                for ch in range(NCH):
                    nc.tensor.matmul(
                        psum_s,
                        lhsT=ws[:, ch, BD:BD + LP],
                        rhs=ws[:, ch, 0:BD],
                        start=(ch == 0),
                        stop=(ch == NCH - 1),
                    )
                ss1 = keep.tile([LP, BD], F32, tag="ss1")
                nc.scalar.mul(out=ss1, in_=psum_s, mul=1.0 / B)
            w = _squash(nc, small, ss1, eps_t, "i0")  # v1; w for iter 2

            vp_pool = ctx.enter_context(
                tc.tile_pool(name="vp", bufs=2, space="PSUM")
            )

            # ---------- iters 2..3 ----------
            for it in range(1, ITERS):
                wB = keep.tile([LP, BD], BF16, tag=f"wB{it}")
                nc.scalar.copy(out=wB, in_=w)
                wBb = wB.unsqueeze(1).to_broadcast([LP, G, BD])
                s_acc = keep.tile([LP, BD], F32, tag=f"sacc{it}")
                nc.vector.memset(s_acc, 0.0)

                def front(g):
                    """PE votes -> ACT cast -> DVE um -> gp t1. Returns (vt, t1)."""
                    wt = load_group(g)
                    vp = vp_pool.tile([LP, G, BD], F32, tag="vp")
                    for j in range(G):
                        nc.tensor.matmul(
                            vp[:, j, :],
                            lhsT=wt[:, j, BD:BD + LP],
                            rhs=wt[:, j, 0:BD],
                            start=True,
                            stop=True,
                        )
                    vt = big.tile([LP, G, BD], BF16, tag="vt")
                    nc.scalar.copy(out=vt, in_=vp)
                    um = big.tile([LP, G, BD], BF16, tag="um")
                    nc.vector.tensor_mul(out=um, in0=vt, in1=wBb)
                    um4 = um.rearrange("p g (d b) -> p g d b", d=D)
                    t1 = big.tile([LP, G, 8, B], BF16, tag="t1")
                    nc.gpsimd.tensor_add(
                        out=t1, in0=um4[:, :, 0:8, :], in1=um4[:, :, 8:16, :]
                    )
                    return vt, t1

                def tail(vt, t1):
                    """finish softmax + weighted sum for a deferred group."""
                    t2 = small.tile([LP, G, 4, B], BF16, tag="t2")
                    nc.vector.tensor_add(
                        out=t2, in0=t1[:, :, 0:4, :], in1=t1[:, :, 4:8, :]
                    )
                    t3 = small.tile([LP, G, 2, B], BF16, tag="t3")
                    nc.vector.tensor_add(
                        out=t3, in0=t2[:, :, 0:2, :], in1=t2[:, :, 2:4, :]
                    )
                    u = small.tile([LP, G, B], BF16, tag="u")
                    nc.vector.tensor_add(
                        out=u, in0=t3[:, :, 0, :], in1=t3[:, :, 1, :]
                    )
                    e = small.tile([LP, G, B], F32, tag="e")
                    nc.scalar.activation(out=e, in_=u, func=ACT.Exp)
                    Z = small.tile([LP, G], F32, tag="Z")
                    nc.vector.tensor_reduce(out=Z, in_=e, axis=AX.X, op=OP.add)
                    rZ = small.tile([LP, G], F32, tag="rZ")
                    nc.vector.reciprocal(out=rZ, in_=Z)
                    c = small.tile([LP, G, B], BF16, tag="c")
                    nc.gpsimd.tensor_mul(
                        out=c, in0=e, in1=rZ.unsqueeze(2).to_broadcast([LP, G, B])
                    )
                    cv = big.tile([LP, G, D, B], BF16, tag="cv")
                    nc.vector.tensor_mul(
                        out=cv,
                        in0=vt.rearrange("p g (d b) -> p g d b", d=D),
                        in1=c.unsqueeze(2).to_broadcast([LP, G, D, B]),
                    )
                    cvf = cv.rearrange("p g d b -> p (g d b)").rearrange(
                        "p (h x) -> p h x", h=2
                    )
                    st = small.tile([LP, 2 * BD], BF16, tag="st")
                    nc.vector.tensor_add(out=st, in0=cvf[:, 0, :], in1=cvf[:, 1, :])
                    sp = small.tile([LP, BD], BF16, tag="sp")
                    nc.vector.tensor_add(
                        out=sp, in0=st[:, 0:BD], in1=st[:, BD:2 * BD]
                    )
                    nc.gpsimd.tensor_add(out=s_acc, in0=s_acc, in1=sp)

                for g in range(NG):
                    tail(*front(g))

                v_it = _squash(nc, sq_pool, s_acc, eps_t, f"i{it}")
                if it < ITERS - 1:
                    w_new = keep.tile([LP, BD], F32, tag=f"w{it}")
                    nc.vector.tensor_add(out=w_new, in0=w, in1=v_it)
                    w = w_new
                else:
                    nc.sync.dma_start(out=vout[:, :], in_=v_it)
    _split_excess_waits(nc)
    return nc


def _host_prep(pose, W):
    """unfold + reorder + shard. returns in_maps."""
    pose = np.asarray(pose, dtype=np.float32)
    W = np.asarray(W, dtype=np.float32)
    b = pose.shape[0]
    cols = np.empty((b, A * C, KK, OH, OW), dtype=np.float32)
    for ki in range(K):
        for kj in range(K):
            cols[:, :, ki * K + kj] = pose[
                :, :, ki:ki + STRIDE * (OH - 1) + 1:STRIDE,
                kj:kj + STRIDE * (OW - 1) + 1:STRIDE,
            ]
    # (b, A, C, KK, l) -> (b, l, KK, A, C) -> (npos, KKA, C)
    p = cols.reshape(b, A, C, KK, L).transpose(0, 4, 3, 1, 2).reshape(
        NPOS, KKA, C
    )
    p_pad = np.zeros((NPOS_PAD, KKA, C), dtype=np.float32)
    p_pad[:NPOS] = p
    # W cols (d,B)-major: Wd[k, c, d*32+B] = W[k, B*16+d, c]
    Wd = W.reshape(KKA, B, D, C).transpose(0, 3, 2, 1).reshape(KKA, C, D * B)
    # dense chunk layout [128=(ksub8,c16), 36ch, 512]
    W2 = Wd.reshape(NCH, 8 * C, BD).transpose(1, 0, 2).astype(ml_dtypes.bfloat16)
    # streaming layout [16c, 288k, 512]
    Wt = Wd.transpose(1, 0, 2).astype(ml_dtypes.bfloat16)
    in_maps = []
    for i in range(NCORES):
        pc = p_pad[i * LP:(i + 1) * LP]                    # [LP, 288, 16]
        P2 = pc.transpose(1, 2, 0).reshape(NCH, 8 * C, LP).transpose(1, 0, 2)
        PW2 = np.empty((LP, NCH, BD + LP), dtype=ml_dtypes.bfloat16)
        PW2[:, :, :BD] = W2
        PW2[:, :, BD:] = P2.astype(ml_dtypes.bfloat16)
        PWs = np.empty((C, KKA, BD + LP), dtype=ml_dtypes.bfloat16)
        PWs[:, :, :BD] = Wt
        PWs[:, :, BD:] = pc.transpose(2, 1, 0).astype(ml_dtypes.bfloat16)
        in_maps.append({"PW": PWs, "PW2": PW2})
    return in_maps


def _gather(results):
    v = np.concatenate([r["vout"] for r in results], axis=0)  # [1024, 512] (d,B)
    v = v[:NPOS].reshape(NB, L, D, B).transpose(0, 3, 2, 1)   # -> (NB, B, D, L)
    return np.ascontiguousarray(
        v.reshape(NB, BD, OH, OW), dtype=np.float32
    )


def _split_excess_waits(nc, max_waits=1):
    """walrus (CoreV2/V3) accepts at most 2 sync-wait commands per
    compute instruction and 1 per DMA; hoist excess waits onto NOPs
    just before, same engine."""
    n_split = 0
    for f in nc.m.functions:
        for bb in f.blocks:
            il = bb.instructions
            out = []
            changed = False
            for inst in il:
                lim = max_waits
                si = inst.sync_info
                if si is not None and si.on_wait and len(si.on_wait) > lim:
                    waits = list(si.on_wait)
                    excess, kept = waits[:-lim], waits[-lim:]
                    for i in range(0, len(excess), max_waits):
                        nop = mybir.InstNoOp(
                            name=f"{inst.name}-w{i}",
                            sync_info=mybir.SyncInfo(
                                on_wait=excess[i:i + max_waits], on_update=[]
                            ),
                            bass_nofuse=True,
                            engine=inst.engine,
                        )
                        out.append(nop)
                        n_split += 1
                    inst.sync_info = mybir.SyncInfo(
                        on_wait=kept, on_update=list(si.on_update or [])
                    )
                    changed = True
                out.append(inst)
            if changed:
                bb.instructions = out
    return n_split


_NC_CACHE = {}


def _get_nc(mm_dtype=None):
    key = "v1"
    if key not in _NC_CACHE:
        _NC_CACHE[key] = _build_nc()
    return _NC_CACHE[key]


def _run(pose, W, trace=False, mm_dtype=None):
    nc = _get_nc(mm_dtype)
    in_maps = _host_prep(pose, W)
    res = run_bass_kernel_spmd(
        nc, in_maps, core_ids=list(range(NCORES)), trace=trace
    )
    return _gather(res.results), res


def kernel(pose, W):
    out, _ = _run(pose, W)
    return out
